# revision 14
# baseline (speedup 1.0000x reference)
"""Trainium2 Bass kernel for nn_EneSc.

reference computation (T=16384, D=4096, QD=256, H=128):
    s        = sum_t E_s[t]                 # [D]
    energy_s = dot(s, s)
    c        = sum_t Att[t] * E_s[t]        # [D]
    energy_c = dot(c, c)
    r        = energy_c / energy_s
    r_th     = sigmoid(W2 @ relu(W1 @ E_q + b1) + b2)
    out      = [r, r_th]

Strategy: data-parallel over T across 8 cores (2048 rows/core). Each core
streams its 32 MiB shard of E_s through SBUF and reduces over the row
(partition) axis with TensorE matmuls using a stationary [128, 2] matrix
[ones | w_block], accumulating into PSUM. Device output per core is
[2, 4096] = (partial sum vector, partial weighted-context vector).
Host sums the 8 partials (the "all-reduce" of two [D] vectors) and runs
the tiny scalar finalize + MLP in numpy.
"""

import numpy as np

from concourse import bacc, mybir, tile
from concourse.bass_utils import run_bass_kernel_spmd

T, D = 16384, 4096
NCORES = 8
RPC = T // NCORES          # rows per core = 2048
P = 128                    # SBUF partitions
NBLK = RPC // P            # 128-row blocks per core = 16
NB = 2                     # row-blocks fetched per DMA (free dim = NB*D)
BUFS = 4                   # SBUF tiles in flight
CHUNK = 512                # matmul free-dim (one PSUM bank of fp32)
NCHUNK = D // CHUNK        # 8

_cached = {}


def _build():
    nc = bacc.Bacc("TRN2", debug=False, num_devices=NCORES)
    f32 = mybir.dt.float32
    # float32r: same fp32 bit layout, but the PE streams it at 1 cycle/row
    # (vs 4 for plain fp32) when the moving free-dim is >=256.
    f32r = mybir.dt.float32r

    e = nc.dram_tensor("e", [RPC, D], f32r, kind="ExternalInput")
    w = nc.dram_tensor("w", [RPC], f32r, kind="ExternalInput")
    o = nc.dram_tensor("o", [2, D], f32, kind="ExternalOutput")

    e_r = e.ap().rearrange("(n p) d -> p n d", p=P)   # [128, 16, 4096]
    w_r = w.ap().rearrange("(n p) -> p n", p=P)       # [128, 16]

    with tile.TileContext(nc) as tc:
        with (
            tc.tile_pool(name="const", bufs=1) as const,
            tc.tile_pool(name="psum", bufs=1, space="PSUM") as psum,
            tc.tile_pool(name="data", bufs=BUFS) as data,
            tc.tile_pool(name="out", bufs=1) as outp,
        ):
            # Every NB-block tile is loaded as two half-D column pieces on
            # alternating HWDGE rings (sync/scalar), and the matmuls chase
            # piece arrival. The final two blocks stream as four 1-MiB
            # quarter-D pieces so almost nothing remains after the last
            # DMA byte. Two rings together reach the SBUF fabric rate
            # (~435 GB/s), above the single-ring rate.
            HD = D // 2
            QD_ = D // 4
            ring = [nc.sync, nc.scalar]

            # Issue the first tile's two half-D DMAs before anything else
            # touches the HWDGE rings so streaming starts immediately.
            tiles = {}
            t = data.tile([P, NB, D], f32r, name="t0", tag="data")
            nc.sync.dma_start(t[:, :, 0:HD], e_r[:, 0:NB, 0:HD])
            nc.scalar.dma_start(t[:, :, HD:D], e_r[:, 0:NB, HD:D])
            tiles[0] = t

            # stationary operand per row-block n: lhs[:, n, :] = [1.0 | w_n].
            # memset can't target f32r, and the BIR verifier requires f32r
            # matmul operands to come from instructions that round to f32r —
            # so memset/DMA into f32 staging, then tensor_copy (f32 -> f32r)
            # which applies the rounding. w loads via SWDGE (gpsimd) to stay
            # off the HWDGE rings that stream E_s.
            w_sb = const.tile([P, NBLK], f32)
            nc.gpsimd.dma_start(w_sb[:], w_r[:, :].bitcast(f32))
            ones_sb = const.tile([P, NBLK], f32)
            nc.gpsimd.memset(ones_sb[:], 1.0)
            lhs = const.tile([P, NBLK, 2], f32r)
            nc.vector.tensor_copy(lhs[:, :, 0], ones_sb[:])
            nc.vector.tensor_copy(lhs[:, :, 1], w_sb[:])

            acc = [
                psum.tile([2, CHUNK], f32, name=f"acc{c}", tag=f"acc{c}")
                for c in range(NCHUNK)
            ]

            nmain = NBLK // NB - 1          # tiles of NB blocks, minus tail
            for i in range(nmain):
                b0 = i * NB
                if i in tiles:
                    t = tiles[i]
                else:
                    t = data.tile([P, NB, D], f32r, name=f"t{i}", tag="data")
                    ring[0].dma_start(t[:, :, 0:HD], e_r[:, b0 : b0 + NB, 0:HD])
                    ring[1].dma_start(t[:, :, HD:D], e_r[:, b0 : b0 + NB, HD:D])
                for h in range(2):
                    for j in range(NB):
                        n = b0 + j
                        for c in range(4 * h, 4 * h + 4):
                            nc.tensor.matmul(
                                acc[c][:],
                                lhs[:, n, :],
                                t[:, j, c * CHUNK : (c + 1) * CHUNK],
                                start=(n == 0),
                                stop=False,
                            )

            # tail: last two blocks as four quarter-D (1 MiB) pieces
            tb = NBLK - 2
            t = data.tile([P, NB, D], f32r, name="t_tail", tag="data")
            for q in range(4):
                ring[q % 2].dma_start(
                    t[:, :, q * QD_ : (q + 1) * QD_],
                    e_r[:, tb : tb + 2, q * QD_ : (q + 1) * QD_],
                )
            o_sb = outp.tile([2, D], f32)
            for q in range(4):
                for c in (2 * q, 2 * q + 1):
                    for j in range(2):
                        n = tb + j
                        nc.tensor.matmul(
                            acc[c][:],
                            lhs[:, n, :],
                            t[:, j, c * CHUNK : (c + 1) * CHUNK],
                            start=False,
                            stop=(n == NBLK - 1),
                        )
                    # drain this chunk as soon as its group closes;
                    # alternate DVE / ACT so the copies pipeline
                    dst = o_sb[:, c * CHUNK : (c + 1) * CHUNK]
                    if c % 2 == 0:
                        nc.vector.tensor_copy(dst, acc[c][:])
                    else:
                        nc.scalar.copy(dst, acc[c][:])

            nc.sync.dma_start(o.ap(), o_sb[:])

    nc.compile()
    return nc


def _get_nc():
    if "nc" not in _cached:
        _cached["nc"] = _build()
    return _cached["nc"]


def _run_device(E_s, Att_weights, **spmd_kwargs):
    nc = _get_nc()
    E_s = np.ascontiguousarray(E_s, dtype=np.float32)
    Att = np.ascontiguousarray(Att_weights, dtype=np.float32)
    in_maps = [
        {"e": E_s[i * RPC : (i + 1) * RPC], "w": Att[i * RPC : (i + 1) * RPC]}
        for i in range(NCORES)
    ]
    res = run_bass_kernel_spmd(nc, in_maps, core_ids=list(range(NCORES)), **spmd_kwargs)
    partials = np.stack([res.results[i]["o"] for i in range(NCORES)])  # [8, 2, D]
    return partials, res


def kernel(E_s, E_q, Att_weights, W1, b1, W2, b2):
    partials, _ = _run_device(E_s, Att_weights)
    s = partials[:, 0, :].astype(np.float64).sum(axis=0)
    c = partials[:, 1, :].astype(np.float64).sum(axis=0)
    energy_s = float(np.dot(s, s))
    energy_c = float(np.dot(c, c))
    r = energy_c / energy_s
    # tiny replicated MLP on E_q (host, ~70k flops)
    h = np.maximum(W1.astype(np.float64) @ E_q.astype(np.float64) + b1, 0.0)
    z = float((W2.astype(np.float64) @ h)[0] + b2[0])
    r_th = 1.0 / (1.0 + np.exp(-z))
    return np.array([r, r_th], dtype=np.float32)


# revision 16
# speedup vs baseline: 1.0023x; 1.0023x over previous
"""Trainium2 Bass kernel for nn_EneSc.

reference computation (T=16384, D=4096, QD=256, H=128):
    s        = sum_t E_s[t]                 # [D]
    energy_s = dot(s, s)
    c        = sum_t Att[t] * E_s[t]        # [D]
    energy_c = dot(c, c)
    r        = energy_c / energy_s
    r_th     = sigmoid(W2 @ relu(W1 @ E_q + b1) + b2)
    out      = [r, r_th]

Strategy: data-parallel over T across 8 cores (2048 rows/core). Each core
streams its 32 MiB shard of E_s through SBUF and reduces over the row
(partition) axis with TensorE matmuls using a stationary [128, 2] matrix
[ones | w_block], accumulating into PSUM. Device output per core is
[2, 4096] = (partial sum vector, partial weighted-context vector).
Host sums the 8 partials (the "all-reduce" of two [D] vectors) and runs
the tiny scalar finalize + MLP in numpy.
"""

import numpy as np

from concourse import bacc, mybir, tile
from concourse.bass_utils import run_bass_kernel_spmd

T, D = 16384, 4096
NCORES = 8
RPC = T // NCORES          # rows per core = 2048
P = 128                    # SBUF partitions
NBLK = RPC // P            # 128-row blocks per core = 16
NB = 2                     # row-blocks fetched per DMA (free dim = NB*D)
BUFS = 4                   # SBUF tiles in flight
CHUNK = 512                # matmul free-dim (one PSUM bank of fp32)
NCHUNK = D // CHUNK        # 8

_cached = {}


def _build():
    nc = bacc.Bacc("TRN2", debug=False, num_devices=NCORES)
    f32 = mybir.dt.float32
    # float32r: same fp32 bit layout, but the PE streams it at 1 cycle/row
    # (vs 4 for plain fp32) when the moving free-dim is >=256.
    f32r = mybir.dt.float32r

    e = nc.dram_tensor("e", [RPC, D], f32r, kind="ExternalInput")
    w = nc.dram_tensor("w", [RPC], f32r, kind="ExternalInput")
    o = nc.dram_tensor("o", [2, D], f32, kind="ExternalOutput")

    e_r = e.ap().rearrange("(n p) d -> p n d", p=P)   # [128, 16, 4096]
    w_r = w.ap().rearrange("(n p) -> p n", p=P)       # [128, 16]

    with tile.TileContext(nc) as tc:
        with (
            tc.tile_pool(name="const", bufs=1) as const,
            tc.tile_pool(name="psum", bufs=1, space="PSUM") as psum,
            tc.tile_pool(name="data", bufs=BUFS) as data,
            tc.tile_pool(name="out", bufs=1) as outp,
        ):
            # Full-D row-block DMAs (contiguous 16 KiB partition lines are
            # the most efficient descriptor shape), alternating between the
            # two HWDGE rings (sync/scalar) — together they reach ~400+ GB/s,
            # above the single-ring rate. Blocks 0..11 go as NB-block tiles;
            # the last 4 blocks go as single-block 2 MiB DMAs so the tail
            # matmuls chase the stream block-by-block.
            ring = [nc.sync, nc.scalar]

            # Issue the first data DMA before anything else touches the
            # HWDGE rings so streaming starts immediately.
            tiles = {}
            t = data.tile([P, NB, D], f32r, name="t0", tag="data")
            nc.sync.dma_start(t[:], e_r[:, 0:NB, :])
            tiles[0] = t

            # stationary operand per row-block n: lhs[:, n, :] = [1.0 | w_n].
            # memset can't target f32r, and the BIR verifier requires f32r
            # matmul operands to come from instructions that round to f32r —
            # so memset/DMA into f32 staging, then tensor_copy (f32 -> f32r)
            # which applies the rounding. w loads via SWDGE (gpsimd) to stay
            # off the HWDGE rings that stream E_s.
            w_sb = const.tile([P, NBLK], f32)
            nc.gpsimd.dma_start(w_sb[:], w_r[:, :].bitcast(f32))
            ones_sb = const.tile([P, NBLK], f32)
            nc.gpsimd.memset(ones_sb[:], 1.0)
            lhs = const.tile([P, NBLK, 2], f32r)
            nc.vector.tensor_copy(lhs[:, :, 0], ones_sb[:])
            nc.vector.tensor_copy(lhs[:, :, 1], w_sb[:])

            acc = [
                psum.tile([2, CHUNK], f32, name=f"acc{c}", tag=f"acc{c}")
                for c in range(NCHUNK)
            ]

            NTAIL = 4                       # trailing single-block DMAs
            nmain = (NBLK - NTAIL) // NB    # leading NB-block tiles
            di = 0                          # ring-alternation counter
            for i in range(nmain):
                b0 = i * NB
                if i in tiles:
                    t = tiles[i]
                else:
                    t = data.tile([P, NB, D], f32r, name=f"t{i}", tag="data")
                    ring[di % 2].dma_start(t[:], e_r[:, b0 : b0 + NB, :])
                di += 1
                for j in range(NB):
                    n = b0 + j
                    for c in range(NCHUNK):
                        nc.tensor.matmul(
                            acc[c][:],
                            lhs[:, n, :],
                            t[:, j, c * CHUNK : (c + 1) * CHUNK],
                            start=(n == 0),
                            stop=False,
                        )

            o_sb = outp.tile([2, D], f32)
            for k in range(NTAIL):
                n = NBLK - NTAIL + k
                t = data.tile([P, NB, D], f32r, name=f"tt{k}", tag="data")
                ring[di % 2].dma_start(t[:, 0, :], e_r[:, n, :])
                di += 1
                last = n == NBLK - 1
                for c in range(NCHUNK):
                    nc.tensor.matmul(
                        acc[c][:],
                        lhs[:, n, :],
                        t[:, 0, c * CHUNK : (c + 1) * CHUNK],
                        start=False,
                        stop=last,
                    )
                    if last:
                        # drain each chunk as soon as its group closes;
                        # alternate DVE / ACT so the copies pipeline
                        dst = o_sb[:, c * CHUNK : (c + 1) * CHUNK]
                        if c % 2 == 0:
                            nc.vector.tensor_copy(dst, acc[c][:])
                        else:
                            nc.scalar.copy(dst, acc[c][:])

            nc.sync.dma_start(o.ap(), o_sb[:])

    nc.compile()
    return nc


def _get_nc():
    if "nc" not in _cached:
        _cached["nc"] = _build()
    return _cached["nc"]


def _run_device(E_s, Att_weights, **spmd_kwargs):
    nc = _get_nc()
    E_s = np.ascontiguousarray(E_s, dtype=np.float32)
    Att = np.ascontiguousarray(Att_weights, dtype=np.float32)
    in_maps = [
        {"e": E_s[i * RPC : (i + 1) * RPC], "w": Att[i * RPC : (i + 1) * RPC]}
        for i in range(NCORES)
    ]
    res = run_bass_kernel_spmd(nc, in_maps, core_ids=list(range(NCORES)), **spmd_kwargs)
    partials = np.stack([res.results[i]["o"] for i in range(NCORES)])  # [8, 2, D]
    return partials, res


def kernel(E_s, E_q, Att_weights, W1, b1, W2, b2):
    partials, _ = _run_device(E_s, Att_weights)
    s = partials[:, 0, :].astype(np.float64).sum(axis=0)
    c = partials[:, 1, :].astype(np.float64).sum(axis=0)
    energy_s = float(np.dot(s, s))
    energy_c = float(np.dot(c, c))
    r = energy_c / energy_s
    # tiny replicated MLP on E_q (host, ~70k flops)
    h = np.maximum(W1.astype(np.float64) @ E_q.astype(np.float64) + b1, 0.0)
    z = float((W2.astype(np.float64) @ h)[0] + b2[0])
    r_th = 1.0 / (1.0 + np.exp(-z))
    return np.array([r, r_th], dtype=np.float32)


# revision 19
# speedup vs baseline: 1.1385x; 1.1358x over previous
"""Trainium2 Bass kernel for nn_EneSc.

reference computation (T=16384, D=4096, QD=256, H=128):
    s        = sum_t E_s[t]                 # [D]
    energy_s = dot(s, s)
    c        = sum_t Att[t] * E_s[t]        # [D]
    energy_c = dot(c, c)
    r        = energy_c / energy_s
    r_th     = sigmoid(W2 @ relu(W1 @ E_q + b1) + b2)
    out      = [r, r_th]

Strategy: data-parallel over T across 8 cores (2048 rows/core). Each core
streams its 32 MiB shard of E_s through SBUF and reduces over the row
(partition) axis with TensorE matmuls using a stationary [128, 2] matrix
[ones | w_block], accumulating into PSUM. Device output per core is
[2, 4096] = (partial sum vector, partial weighted-context vector).
Host sums the 8 partials (the "all-reduce" of two [D] vectors) and runs
the tiny scalar finalize + MLP in numpy.
"""

import numpy as np

from concourse import bacc, mybir, tile
from concourse.bass_utils import run_bass_kernel_spmd

T, D = 16384, 4096
NCORES = 8
RPC = T // NCORES          # rows per core = 2048
P = 128                    # SBUF partitions
NBLK = RPC // P            # 128-row blocks per core = 16
BUFS = 8                   # SBUF data tiles in flight (8 x 16KB/partition)
CHUNK = 512                # matmul free-dim (one PSUM bank of fp32)
NCHUNK = D // CHUNK        # 8

_cached = {}


def _build():
    nc = bacc.Bacc("TRN2", debug=False, num_devices=NCORES)
    f32 = mybir.dt.float32
    # float32r: same fp32 bit layout, but the PE streams it at 1 cycle/row
    # (vs 4 for plain fp32) when the moving free-dim is >=256.
    f32r = mybir.dt.float32r

    e = nc.dram_tensor("e", [RPC, D], f32r, kind="ExternalInput")
    w = nc.dram_tensor("w", [RPC], f32r, kind="ExternalInput")
    o = nc.dram_tensor("o", [2, D], f32, kind="ExternalOutput")

    e_r = e.ap().rearrange("(n p) d -> p n d", p=P)   # [128, 16, 4096]
    w_r = w.ap().rearrange("(n p) -> p n", p=P)       # [128, 16]

    with tile.TileContext(nc) as tc:
        with (
            tc.tile_pool(name="const", bufs=1) as const,
            tc.tile_pool(name="psum", bufs=1, space="PSUM") as psum,
            tc.tile_pool(name="data", bufs=BUFS) as data,
            tc.tile_pool(name="out", bufs=1) as outp,
        ):
            # Full-D single-block DMAs (contiguous 16 KiB partition lines are
            # the most efficient descriptor shape), alternating between the
            # two HWDGE rings (sync/scalar) — together they reach ~430 GB/s
            # (SBUF fabric bound), above the ~345 single-ring rate. bufs=8
            # keeps both ring FIFOs deeply prefetched, and the matmuls chase
            # the stream block-by-block so almost nothing remains after the
            # last DMA byte.
            ring = [nc.sync, nc.scalar]

            # Issue the first data DMA before anything else touches the
            # HWDGE rings so streaming starts immediately.
            tiles = {}
            t = data.tile([P, D], f32r, name="t0", tag="data")
            nc.sync.dma_start(t[:], e_r[:, 0, :])
            tiles[0] = t

            # stationary operand per row-block n: lhs[:, n, :] = [1.0 | w_n].
            # memset can't target f32r, and the BIR verifier requires f32r
            # matmul operands to come from instructions that round to f32r —
            # so memset/DMA into f32 staging, then tensor_copy (f32 -> f32r)
            # which applies the rounding. w loads via SWDGE (gpsimd) to stay
            # off the HWDGE rings that stream E_s.
            w_sb = const.tile([P, NBLK], f32)
            nc.gpsimd.dma_start(w_sb[:], w_r[:, :].bitcast(f32))
            ones_sb = const.tile([P, NBLK], f32)
            nc.gpsimd.memset(ones_sb[:], 1.0)
            lhs = const.tile([P, NBLK, 2], f32r)
            nc.vector.tensor_copy(lhs[:, :, 0], ones_sb[:])
            nc.vector.tensor_copy(lhs[:, :, 1], w_sb[:])

            acc = [
                psum.tile([2, CHUNK], f32, name=f"acc{c}", tag=f"acc{c}")
                for c in range(NCHUNK)
            ]

            o_sb = outp.tile([2, D], f32)
            for n in range(NBLK):
                if n in tiles:
                    t = tiles[n]
                else:
                    t = data.tile([P, D], f32r, name=f"t{n}", tag="data")
                    ring[n % 2].dma_start(t[:], e_r[:, n, :])
                last = n == NBLK - 1
                for c in range(NCHUNK):
                    nc.tensor.matmul(
                        acc[c][:],
                        lhs[:, n, :],
                        t[:, c * CHUNK : (c + 1) * CHUNK],
                        start=(n == 0),
                        stop=last,
                    )
                    if last:
                        # drain each chunk as soon as its group closes;
                        # alternate DVE / ACT so the copies pipeline
                        dst = o_sb[:, c * CHUNK : (c + 1) * CHUNK]
                        if c % 2 == 0:
                            nc.vector.tensor_copy(dst, acc[c][:])
                        else:
                            nc.scalar.copy(dst, acc[c][:])

            nc.sync.dma_start(o.ap(), o_sb[:])

    nc.compile()
    return nc


def _get_nc():
    if "nc" not in _cached:
        _cached["nc"] = _build()
    return _cached["nc"]


def _run_device(E_s, Att_weights, **spmd_kwargs):
    nc = _get_nc()
    E_s = np.ascontiguousarray(E_s, dtype=np.float32)
    Att = np.ascontiguousarray(Att_weights, dtype=np.float32)
    in_maps = [
        {"e": E_s[i * RPC : (i + 1) * RPC], "w": Att[i * RPC : (i + 1) * RPC]}
        for i in range(NCORES)
    ]
    res = run_bass_kernel_spmd(nc, in_maps, core_ids=list(range(NCORES)), **spmd_kwargs)
    partials = np.stack([res.results[i]["o"] for i in range(NCORES)])  # [8, 2, D]
    return partials, res


def kernel(E_s, E_q, Att_weights, W1, b1, W2, b2):
    partials, _ = _run_device(E_s, Att_weights)
    s = partials[:, 0, :].astype(np.float64).sum(axis=0)
    c = partials[:, 1, :].astype(np.float64).sum(axis=0)
    energy_s = float(np.dot(s, s))
    energy_c = float(np.dot(c, c))
    r = energy_c / energy_s
    # tiny replicated MLP on E_q (host, ~70k flops)
    h = np.maximum(W1.astype(np.float64) @ E_q.astype(np.float64) + b1, 0.0)
    z = float((W2.astype(np.float64) @ h)[0] + b2[0])
    r_th = 1.0 / (1.0 + np.exp(-z))
    return np.array([r, r_th], dtype=np.float32)
